# revision 35
# baseline (speedup 1.0000x reference)
"""Memory-augmented forecaster kernel for 8 Trainium2 NeuronCores.

Pipeline (3 SPMD launches; host does only sharding/layout/merge between):
  L1 (batch-sharded, 32 queries/core): hid streamed as fp16 in 2MB group
      DMAs (host converts once; halves HBM traffic); per batch one DVE fold
      (fp16, 2x mode) + 2 accumulating ones-matmuls on PE give the exact
      fp32 series mean; q = series/|series| in fp32.  DMA-bound ~47us.
  L2 (bank-sharded, 12500 rows/core): sims = q8 @ bank8_shard.T as fp8e4
      DoubleRow PE matmuls (0.5 cyc/row, 2 contraction rows/partition);
      per 1024-column tile the DVE max/max_index ops (fp16 values, u16
      indices -> 2-byte fast mode) return that tile's raw top-8 -> 104
      candidates/query/core.  The fp8 sims only steer candidate *selection*;
      the survivors are rescored exactly on host (rel err 2.15e-3 measured).
  host: pool the 8x104 candidates/query, take top-32 by noisy value,
      rescore exactly in fp32, re-filter threshold/exclude-self, top-16,
      gather retrieved rows from the fp16 bank (pure layout work).
  L3 (batch-sharded): packed small inputs (1 weight blob, 1 seriesT|retrT
      blob, 1 per-row vector blob) on the ACT DMA queue; hid group-loads on
      the SP queue; gated cross-attention over the top-16 with fp16
      weights/rows (fp32 PSUM accumulate), wretT via a DVE weighted
      reduction of retrT, gating, LayerNorm; all 32 delta rows are
      replicated+downcast to fp16 up front, then out = hid + delta runs as
      load(2MB, SP) -> 4x DVE fp16 add -> store(2MB, ACT) fully pipelined.
      Host upcasts the fp16 output to fp32.
"""

import os
import numpy as np
import ml_dtypes

import concourse.bacc as bacc
import concourse.mybir as mybir
from concourse import bass_utils
from concourse.tile import TileContext
from concourse.masks import make_identity

F32 = mybir.dt.float32
F16 = mybir.dt.float16
F8 = mybir.dt.float8e4
U16 = mybir.dt.uint16
AX = mybir.AxisListType
OP = mybir.AluOpType
ACT = mybir.ActivationFunctionType
DR = mybir.MatmulPerfMode.DoubleRow
FP8 = ml_dtypes.float8_e4m3

B, S, D = 256, 512, 512
M, TOPK = 100000, 16
NC = 8
BL = B // NC          # 32 queries per core (L1/L3)
ML = M // NC          # 12500 bank rows per core (L2)
CT = 4096             # L2 column tile (folded 8->512 before top-8)
# short tile emitted first so its selection overlaps the first big load
L2_EMIT = ([((ML // CT) * CT, ML % CT)] if ML % CT else []) + [
    (i * CT, CT) for i in range(ML // CT)]
NCAND = 8 * len(L2_EMIT)    # per-core candidates: top-8 per column tile
POOL = 24             # host rescore pool (slots) per query
SCALE = D ** -0.5
LN_EPS = 1e-5
GATE_TEMP = 1.0
THRESH = 0.0
NEG = -1.0e38
GB = 4                # batches per hid DMA group (L1/L3)
N_ST = S // 128       # 4

EXEC_NS = {}

_programs = {}


# ---------------------------------------------------------------- L1 -----
def _build_l1():
    nc = bacc.Bacc("TRN2", target_bir_lowering=False, debug=False)
    hid = nc.dram_tensor("hid", (BL, S, D), F16, kind="ExternalInput").ap()
    series_o = nc.dram_tensor("series", (BL, D), F32, kind="ExternalOutput").ap()

    with TileContext(nc) as tc:
        with (
            tc.tile_pool(name="hidp", bufs=3) as hidp,
            tc.tile_pool(name="cst", bufs=1) as cst,
            tc.tile_pool(name="sml", bufs=1) as sml,
            tc.tile_pool(name="ps", bufs=4, space="PSUM") as psp,
        ):
            ones = cst.tile([128, 1], F16)
            nc.vector.memset(ones[:, :], 1.0)
            seriesF = sml.tile([1, BL * D], F32)
            G1 = 2  # batches per load: fast pipeline start, SEQ keeps ahead
            for g in range(BL // G1):
                t = hidp.tile([128, G1, N_ST, D], F16, tag="hload", bufs=5)
                nc.sync.dma_start(
                    t[:, :, :, :],
                    hid[g * G1:(g + 1) * G1]
                    .rearrange("b (st p) d -> p b st d", p=128))
                for bb in range(G1):
                    b = g * G1 + bb
                    # fold the s-subtiles on DVE (fp16 2x mode, pair sums
                    # exact to ~2^-11) so PE issues only one matmul/batch —
                    # the PE sequencer is the scarce resource here
                    u = hidp.tile([128, 2, D], F16, tag="ufold", bufs=4)
                    nc.vector.tensor_add(
                        u[:, :, :], t[:, bb, 0:2, :], t[:, bb, 2:4, :])
                    v = hidp.tile([128, D], F16, tag="vfold", bufs=4)
                    nc.vector.tensor_add(v[:, :], u[:, 0, :], u[:, 1, :])
                    ps = psp.tile([1, D], F32, tag="pser")
                    nc.tensor.matmul(
                        ps[:, :], ones[:, :], v[:, :], start=True, stop=True)
                    nc.scalar.activation(
                        seriesF[0:1, b * D:(b + 1) * D], ps[:, :], ACT.Copy,
                        scale=1.0 / S)
            # q = series/|series| happens on host (O(B*D) glue next to the
            # merge); the only tail here is one output DMA
            nc.sync.dma_start(
                series_o.rearrange("b d -> (b d)")[None, :], seriesF[:, :])
    nc.compile()
    return nc


# ---------------------------------------------------------------- L2 -----
def _build_l2():
    nc = bacc.Bacc("TRN2", target_bir_lowering=False, debug=False)
    qT = nc.dram_tensor("qT", (D, B), F8, kind="ExternalInput").ap()
    bankT = nc.dram_tensor("bankT", (D, ML), F8, kind="ExternalInput").ap()
    tv_o = nc.dram_tensor("tv", (B, NCAND), F16, kind="ExternalOutput").ap()
    ti_o = nc.dram_tensor("ti", (B, NCAND), U16, kind="ExternalOutput").ap()

    JD = D // 256  # 2 DoubleRow contraction steps (256 rows each)

    with TileContext(nc) as tc:
        with (
            tc.tile_pool(name="qp", bufs=1) as qp,
            tc.tile_pool(name="bkp", bufs=4) as bkp,
            tc.tile_pool(name="stg", bufs=4) as stg,
            tc.tile_pool(name="outp", bufs=1) as outp,
            tc.tile_pool(name="ps", bufs=3, space="PSUM") as psp,
        ):
            qt = qp.tile([128, JD, 2, B], F8)
            nc.scalar.dma_start(
                qt[:, :, :, :],
                qT.rearrange("(j t p) b -> p j t b", p=128, t=2))
            vals = [outp.tile([128, NCAND], F16, tag=f"v{blk}",
                              name=f"v{blk}") for blk in range(2)]
            idxs = [outp.tile([128, NCAND], U16, tag=f"i{blk}",
                              name=f"i{blk}") for blk in range(2)]
            for t, (c0t, cw) in enumerate(L2_EMIT):
                bk = bkp.tile([128, JD, 2, CT], F8, tag="bk", bufs=2)
                nc.sync.dma_start(
                    bk[:, :, :, :cw],
                    bankT.rearrange("(j t p) c -> p j t c", p=128, t=2)
                    [:, :, :, c0t:c0t + cw])
                for blk in range(2):
                    sl = slice(t * 8, t * 8 + 8)
                    if cw == CT:
                        # 8 sims chunks per block in 4 psum pairs (depth-4
                        # pipeline across PE copy->fold stages).  One ACT
                        # copy downcasts each 1024-col pair; Max/MaxIndex get
                        # no 2-byte speedup, so a pure-fp16 DVE max-tree
                        # folds 8->1 first.  The top-8 of the folded 512
                        # keeps pos%512; the host expands each slot to its 8
                        # possible rows before exact rescoring.
                        nas = []
                        for pr in range(4):
                            pp = psp.tile([128, 2, 512], F32, tag="ps",
                                          bufs=4)
                            for half in range(2):
                                cc = pr * 1024 + half * 512
                                for j in range(JD):
                                    nc.tensor.matmul(
                                        pp[:, half, :],
                                        qt[:, j, :, blk * 128:(blk + 1) * 128],
                                        bk[:, j, :, cc:cc + 512],
                                        start=(j == 0), stop=(j == JD - 1),
                                        perf_mode=DR,
                                    )
                            st = stg.tile([128, 1024], F16, tag=f"st{blk}",
                                          bufs=6)
                            nc.scalar.copy(st[:, :], pp[:, :, :])
                            na = stg.tile([128, 512], F16, tag=f"na{blk}",
                                          bufs=5)
                            nc.vector.tensor_max(
                                na[:, :], st[:, 0:512], st[:, 512:1024])
                            nas.append(na)
                        nb0 = stg.tile([128, 512], F16, tag=f"nb{blk}", bufs=2)
                        nc.vector.tensor_max(nb0[:, :], nas[0][:, :], nas[1][:, :])
                        nb1 = stg.tile([128, 512], F16, tag=f"nc{blk}", bufs=2)
                        nc.vector.tensor_max(nb1[:, :], nas[2][:, :], nas[3][:, :])
                        m2 = stg.tile([128, 512], F16, tag=f"mf{blk}")
                        nc.vector.tensor_max(m2[:, :], nb0[:, :], nb1[:, :])
                        nc.vector.max(vals[blk][:, sl], m2[:, :])
                        nc.vector.max_index(idxs[blk][:, sl],
                                            vals[blk][:, sl], m2[:, :])
                    else:
                        pt = psp.tile([128, 2, 512], F32, tag="ps", bufs=4)
                        for j in range(JD):
                            nc.tensor.matmul(
                                pt[:, 0, :cw],
                                qt[:, j, :, blk * 128:(blk + 1) * 128],
                                bk[:, j, :, :cw],
                                start=(j == 0), stop=(j == JD - 1),
                                perf_mode=DR,
                            )
                        st = stg.tile([128, 1024], F16, tag=f"st{blk}", bufs=6)
                        nc.scalar.copy(st[:, :cw], pt[:, 0, :cw])
                        nc.vector.max(vals[blk][:, sl], st[:, :cw])
                        nc.vector.max_index(idxs[blk][:, sl],
                                            vals[blk][:, sl], st[:, :cw])
            for blk in range(2):
                nc.scalar.dma_start(tv_o[blk * 128:(blk + 1) * 128, :],
                                    vals[blk][:, :])
                nc.scalar.dma_start(ti_o[blk * 128:(blk + 1) * 128, :],
                                    idxs[blk][:, :])
    nc.compile()
    return nc


# ---------------------------------------------------------------- L3 -----
def _build_l3():
    nc = bacc.Bacc("TRN2", target_bir_lowering=False, debug=False)
    hid = nc.dram_tensor("hid", (BL, S, D), F16, kind="ExternalInput").ap()
    wall = nc.dram_tensor("wall", (4 * D, D), F16, kind="ExternalInput").ap()
    strT = nc.dram_tensor("strT", (D, BL + BL * TOPK), F16, kind="ExternalInput").ap()
    series_i = nc.dram_tensor("series", (BL, D), F32, kind="ExternalInput").ap()
    vecp = nc.dram_tensor("vecp", (BL, 5, D), F16, kind="ExternalInput").ap()
    tiny = nc.dram_tensor("tiny", (BL, TOPK + 1), F32, kind="ExternalInput").ap()
    rowp = nc.dram_tensor("rowp", (2 * BL * TOPK,), F32, kind="ExternalInput").ap()
    bqk = nc.dram_tensor("bqk", (128, 12), F32, kind="ExternalInput").ap()
    out_o = nc.dram_tensor("out", (BL, S, D), F16, kind="ExternalOutput").ap()

    J = D // 128  # 4
    R = BL * TOPK  # 512 retrieved rows

    with TileContext(nc) as tc:
        with (
            tc.tile_pool(name="wp", bufs=1) as wp,
            tc.tile_pool(name="act", bufs=1) as actp,
            tc.tile_pool(name="sml", bufs=1) as sml,
            tc.tile_pool(name="hidp", bufs=5) as hidp,
            tc.tile_pool(name="psA", bufs=2, space="PSUM") as psA,
        ):
            # Small front inputs FIRST on the SP queue, then the hid group
            # loads behind them — both share one DMA resource and the smalls
            # gate the whole attention chain; letting 16MB of hid loads win
            # the arbitration serializes reads before writes (+40us).
            # Stores get the ACT queue to themselves.
            w_t = wp.tile([128, 4, J, D], F16)
            nc.sync.dma_start(
                w_t[:, :, :, :], wall.rearrange("(w j p) e -> p w j e", p=128, j=J))
            sr_t = wp.tile([128, J, BL + R], F16)
            nc.sync.dma_start(
                sr_t[:, :, :], strT.rearrange("(j p) x -> p j x", p=128))
            series_t = sml.tile([BL, D], F32)
            nc.sync.dma_start(series_t[:, :], series_i)
            vec_t = sml.tile([BL, 5, D], F16)
            nc.sync.dma_start(vec_t[:, :, :], vecp)
            tiny_t = sml.tile([BL, TOPK + 1], F32)
            nc.sync.dma_start(tiny_t[:, :], tiny)
            rowp_t = sml.tile([1, 2 * BL * TOPK], F32)
            nc.sync.dma_start(rowp_t[:, :], rowp[None, :])
            bqk_t = sml.tile([128, 12], F32)
            nc.sync.dma_start(bqk_t[:, :], bqk)

            wq, wk, wv, wo = (w_t[:, i] for i in range(4))
            st_t = sr_t[:, :, :BL]
            rt_t = sr_t[:, :, BL:]
            series = series_t[:, :]
            bo_rep = vec_t[:, 0]
            wgs_rep = vec_t[:, 1]
            wgm_rep = vec_t[:, 2]
            lng_rep = vec_t[:, 3]
            lnb_rep = vec_t[:, 4]
            topv = tiny_t[:, :TOPK]
            bg_t = tiny_t[:, TOPK:TOPK + 1]
            bqT = bqk_t[:, 0:J]
            bkT = bqk_t[:, J:2 * J]
            bvT = bqk_t[:, 2 * J:3 * J]

            hts = []
            for g in range(BL // GB):
                # all 8 groups stay resident (16MB) so the load stream never
                # stalls on the attention chain
                ht = hidp.tile([128, GB, N_ST, D], F16, tag="hload", bufs=6)
                nc.sync.dma_start(
                    ht[:, :, :, :],
                    hid[g * GB:(g + 1) * GB]
                    .rearrange("b (st p) d -> p b st d", p=128))
                hts.append(ht)

            ones128 = sml.tile([128, 1], F32)
            nc.vector.memset(ones128[:, :], 1.0)
            ones_row = sml.tile([1, 128], F32)
            nc.vector.memset(ones_row[:, :], 1.0)
            # keep the PE p-state ramped through the load phase so the
            # attention matmuls run at full speed when their data lands
            warm = sml.tile([128, 128], F32)
            nc.vector.memset(warm[:, :], 0.0)
            pwarm = psA.tile([128, 64], F32, tag="pd", bufs=1)
            for _ in range(48):
                nc.tensor.matmul(pwarm[:, :], warm[:, :], warm[:, 0:64],
                                 start=True, stop=True)

            # ---- ops that depend only on early small loads (off the
            # ---- attention critical path: hoisted to overlap the matmuls)
            maxsim = sml.tile([BL, 1], F32)
            nc.vector.tensor_reduce(maxsim[:, :], topv, axis=AX.X, op=OP.max)
            scr = sml.tile([BL, D], F32, tag="tmpbd", bufs=2)
            a1 = sml.tile([BL, 1], F32)
            nc.vector.scalar_tensor_tensor(
                out=scr[:, :], in0=series, scalar=1.0, in1=wgs_rep,
                op0=OP.mult, op1=OP.mult, accum_out=a1[:, :])
            lnb_ms = sml.tile([BL, D], F32)
            nc.vector.tensor_sub(lnb_ms[:, :], lnb_rep, series)

            # QpT[e, b] = sum_d WqT[d, e] seriesT[d, b]  (+bq per-partition e)
            qpT = actp.tile([128, J, BL], F32, tag="qpT")
            for eb in range(J):
                pq = psA.tile([128, BL], F32, tag="smallmm")
                for dj in range(J):
                    nc.tensor.matmul(
                        pq[:, :], wq[:, dj, eb * 128:(eb + 1) * 128],
                        st_t[:, dj, :], start=(dj == 0), stop=(dj == J - 1))
                nc.vector.tensor_scalar(
                    qpT[:, eb, :], pq[:, :], bqT[:, eb:eb + 1], None, op0=OP.add)

            # scores[b, k] = SCALE * sum_e QpT[e, b] KpT[e, b*16+k]; the
            # per-eb partials go straight into an accumulating ones-matmul
            # (no DVE reduction chain)
            psc = psA.tile([1, R], F32, tag="smallmm")
            for eb in range(J):
                pk = psA.tile([128, R], F32, tag="big")
                for dj in range(J):
                    nc.tensor.matmul(
                        pk[:, :], wk[:, dj, eb * 128:(eb + 1) * 128],
                        rt_t[:, dj, :], start=(dj == 0), stop=(dj == J - 1))
                qbc = (qpT[:, eb, :][:, :, None]
                       .to_broadcast([128, BL, TOPK]))
                prod_c = actp.tile([128, R], F32, tag="prodc", bufs=2)
                nc.vector.scalar_tensor_tensor(
                    out=prod_c[:, :].rearrange("p (b k) -> p b k", k=TOPK),
                    in0=pk[:, :].rearrange("p (b k) -> p b k", k=TOPK),
                    scalar=bkT[:, eb:eb + 1], in1=qbc,
                    op0=OP.add, op1=OP.mult)
                nc.tensor.matmul(
                    psc[:, :], ones128[:, :], prod_c[:, :],
                    start=(eb == 0), stop=(eb == J - 1))
            # The whole softmax runs in the [1, R] row layout on one
            # partition (<=512-el DVE/ACT ops), so no SBUF->SBUF reshape
            # DMAs sit on the attention critical path.  pen/mask rows come
            # packed from the host.
            pen_row = rowp_t[0:1, 0:R]
            mask_row = rowp_t[0:1, R:2 * R]
            sc_row = sml.tile([1, R], F32)
            nc.vector.scalar_tensor_tensor(
                out=sc_row[:, :], in0=psc[0:1, :], scalar=SCALE,
                in1=pen_row, op0=OP.mult, op1=OP.add)
            gmax = sml.tile([1, BL], F32)
            nc.vector.tensor_reduce(
                gmax[:, :], sc_row[:, :].rearrange("p (b k) -> p b k", k=TOPK),
                axis=AX.X, op=OP.max)
            shifted = sml.tile([1, R], F32)
            nc.vector.tensor_sub(
                shifted[:, :].rearrange("p (b k) -> p b k", k=TOPK),
                sc_row[:, :].rearrange("p (b k) -> p b k", k=TOPK),
                gmax[:, :, None].to_broadcast([1, BL, TOPK]))
            ex_row = sml.tile([1, R], F32)
            nc.scalar.activation(ex_row[:, :], shifted[:, :], ACT.Exp)
            em_row = sml.tile([1, R], F32)
            nc.vector.tensor_mul(em_row[:, :], ex_row[:, :], mask_row)
            den_row = sml.tile([1, BL], F32)
            nc.vector.tensor_reduce(
                den_row[:, :], em_row[:, :].rearrange("p (b k) -> p b k", k=TOPK),
                axis=AX.X, op=OP.add)
            rden_row = sml.tile([1, BL], F32)
            nc.vector.reciprocal(rden_row[:, :], den_row[:, :])
            attn_row = sml.tile([1, R], F32)
            nc.vector.tensor_mul(
                attn_row[:, :].rearrange("p (b k) -> p b k", k=TOPK),
                em_row[:, :].rearrange("p (b k) -> p b k", k=TOPK),
                rden_row[:, :, None].to_broadcast([1, BL, TOPK]))

            # wretT[d, j, b] = sum_k retrT[d, j, b*K+k] * attn[b, k]:
            # replicate attn as a [1, R] row across 128 partitions via a
            # ones-matmul, then one fused DVE multiply over all J and one
            # innermost-k reduction.
            pab = psA.tile([128, R], F32, tag="big")
            nc.tensor.matmul(
                pab[:, :], ones_row[0:1, :], attn_row[0:1, :],
                start=True, stop=True)
            prodw = actp.tile([128, J, R], F16, tag="prodw")
            nc.vector.tensor_mul(
                prodw[:, :, :], rt_t[:, :, :],
                pab[:, None, :].to_broadcast([128, J, R]))
            wretF = actp.tile([128, J, BL], F32, tag="wretF")
            nc.vector.tensor_reduce(
                wretF[:, :, :],
                prodw[:, :, :].rearrange("p j (b k) -> p (j b) k", k=TOPK),
                axis=AX.X, op=OP.add)
            wretT = actp.tile([128, J, BL], F16, tag="wretT")
            nc.scalar.copy(wretT[:, :, :], wretF[:, :, :])

            # mem_outT[e, b] = sum_d wret[b, d] WvT[d, e] + bv[e], computed
            # directly in the transposed layout (no PE transpose round-trip)
            mvT = actp.tile([128, J, BL], F16, tag="mvT")
            for eb in range(J):
                pmv = psA.tile([128, BL], F32, tag="smallmm")
                for dj in range(J):
                    nc.tensor.matmul(
                        pmv[:, :], wv[:, dj, eb * 128:(eb + 1) * 128],
                        wretT[:, dj, :], start=(dj == 0), stop=(dj == J - 1))
                nc.vector.tensor_scalar(
                    mvT[:, eb, :], pmv[:, :], bvT[:, eb:eb + 1], None, op0=OP.add)

            # mo2[b, f] = (mem_out @ WoT)[b, f] + bo[f]
            pmo2 = psA.tile([BL, D], F32, tag="big")
            for j in range(J):
                nc.tensor.matmul(
                    pmo2[:, :], mvT[:, j, :], wo[:, j, :],
                    start=(j == 0), stop=(j == J - 1))
            mo2 = sml.tile([BL, D], F32)
            nc.vector.tensor_add(mo2[:, :], pmo2[:, :], bo_rep)

            # gate = sigmoid(series.wgs + mo2.wgm + bg); conf hoisted above
            scr2 = sml.tile([BL, D], F32, tag="tmpbd", bufs=2)
            a2 = sml.tile([BL, 1], F32)
            nc.vector.scalar_tensor_tensor(
                out=scr2[:, :], in0=mo2[:, :], scalar=1.0, in1=wgm_rep,
                op0=OP.mult, op1=OP.mult, accum_out=a2[:, :])
            gsum2 = sml.tile([BL, 1], F32)
            nc.vector.scalar_tensor_tensor(
                out=gsum2[:, :], in0=a1[:, :], scalar=bg_t, in1=a2[:, :],
                op0=OP.add, op1=OP.add)
            gate = sml.tile([BL, 1], F32)
            nc.scalar.activation(gate[:, :], gsum2[:, :], ACT.Sigmoid)
            conf = sml.tile([BL, 1], F32)
            nc.scalar.activation(conf[:, :], maxsim[:, :], ACT.Sigmoid)
            gc = sml.tile([BL, 1], F32)
            nc.vector.tensor_mul(gc[:, :], gate[:, :], conf[:, :])
            fused = sml.tile([BL, D], F32)
            nc.vector.scalar_tensor_tensor(
                out=fused[:, :], in0=mo2[:, :], scalar=gc[:, 0:1],
                in1=series, op0=OP.mult, op1=OP.add)

            # LayerNorm; delta = LN(fused) - series folds the final +lnb and
            # -series into one precomputed lnb_ms operand
            fsum = sml.tile([BL, 1], F32)
            nc.vector.tensor_reduce(fsum[:, :], fused[:, :], axis=AX.X, op=OP.add)
            mu = sml.tile([BL, 1], F32)
            nc.vector.tensor_scalar(mu[:, :], fsum[:, :], 1.0 / D, None, op0=OP.mult)
            xc = sml.tile([BL, D], F32)
            nc.vector.tensor_scalar(xc[:, :], fused[:, :], mu[:, 0:1], None, op0=OP.subtract)
            sq = sml.tile([BL, D], F32, tag="tmpbd", bufs=2)
            vs = sml.tile([BL, 1], F32)
            nc.vector.scalar_tensor_tensor(
                out=sq[:, :], in0=xc[:, :], scalar=1.0, in1=xc[:, :],
                op0=OP.mult, op1=OP.mult, accum_out=vs[:, :])
            varp = sml.tile([BL, 1], F32)
            nc.vector.tensor_scalar(
                varp[:, :], vs[:, :], 1.0 / D, LN_EPS, op0=OP.mult, op1=OP.add)
            sd = sml.tile([BL, 1], F32)
            nc.scalar.sqrt(sd[:, :], varp[:, :])
            rsd = sml.tile([BL, 1], F32)
            nc.vector.reciprocal(rsd[:, :], sd[:, :])
            xng = sml.tile([BL, D], F32, tag="tmpbd", bufs=2)
            nc.vector.scalar_tensor_tensor(
                out=xng[:, :], in0=xc[:, :], scalar=rsd[:, 0:1], in1=lng_rep,
                op0=OP.mult, op1=OP.mult)
            delta = sml.tile([BL, D], F32)
            nc.vector.tensor_add(delta[:, :], xng[:, :], lnb_ms[:, :])

            # Replicate every delta row across 128 partitions (group
            # ones-matmuls into one 4-bank psum tile + one fp16 downcast
            # copy per group) BEFORE any store issues, so the ACT queue
            # never head-of-line blocks.
            delta16 = sml.tile([BL, D], F16)
            nc.scalar.copy(delta16[:, :], delta[:, :])
            ones_r16 = sml.tile([1, 128], F16)
            nc.vector.memset(ones_r16[:, :], 1.0)
            # out[b, s, :] = hid[b, s, :] + delta[b, :], pipelined per
            # group: stage 4 delta rows (SP), replicate via ones-matmul,
            # downcast (ACT), one fused DVE add, 2MB store (ACT).
            for g in range(BL // GB):
                dF = sml.tile([1, GB * D], F16, tag="dF", bufs=2)
                nc.sync.dma_start(dF[:, :], delta16[g * GB:(g + 1) * GB, :])
                pd = psA.tile([128, GB, D], F32, tag="pd", bufs=1)
                for bb in range(GB):
                    nc.tensor.matmul(
                        pd[:, bb, :], ones_r16[0:1, :],
                        dF[0:1, bb * D:(bb + 1) * D], start=True, stop=True)
                dS = sml.tile([128, GB, D], F16, tag="dS", bufs=2)
                nc.scalar.copy(dS[:, :, :], pd[:, :, :])
                ht = hts[g]
                nc.vector.tensor_add(
                    ht[:, :, :, :], ht[:, :, :, :],
                    dS[:, :, None, :].to_broadcast([128, GB, N_ST, D]))
                nc.scalar.dma_start(
                    out_o[g * GB:(g + 1) * GB]
                    .rearrange("b (st p) d -> p b st d", p=128),
                    ht[:, :, :, :])
    nc.compile()
    return nc


def _get(name):
    if name not in _programs:
        _programs[name] = {"l1": _build_l1, "l2": _build_l2, "l3": _build_l3}[name]()
    return _programs[name]


def _run(nc, in_maps, tag):
    trace = os.environ.get("KNN_TRACE") == "1"
    res = bass_utils.run_bass_kernel_spmd(
        nc, in_maps, core_ids=list(range(NC)), trace=trace)
    if trace:
        EXEC_NS[tag] = res.exec_time_ns
    return res.results


def kernel(**inputs):
    hs = np.ascontiguousarray(np.asarray(inputs["hidden_states"], np.float32))
    mb = np.ascontiguousarray(np.asarray(inputs["memory_bank"], np.float32))
    Wq, bq = np.asarray(inputs["Wq"], np.float32), np.asarray(inputs["bq"], np.float32)
    Wk, bk = np.asarray(inputs["Wk"], np.float32), np.asarray(inputs["bk"], np.float32)
    Wv, bv = np.asarray(inputs["Wv"], np.float32), np.asarray(inputs["bv"], np.float32)
    Wo, bo = np.asarray(inputs["Wo"], np.float32), np.asarray(inputs["bo"], np.float32)
    Wg, bg = np.asarray(inputs["Wg"], np.float32), np.asarray(inputs["bg"], np.float32)
    ln_g, ln_b = np.asarray(inputs["ln_g"], np.float32), np.asarray(inputs["ln_b"], np.float32)

    hs16 = hs.astype(np.float16)

    # ---- L1: series + normalized query, batch-sharded ----
    l1 = _get("l1")
    r1 = _run(l1, [{"hid": hs16[i * BL:(i + 1) * BL]} for i in range(NC)], "l1")
    series = np.concatenate([r1[i]["series"] for i in range(NC)], axis=0)
    q = series / np.linalg.norm(series, axis=-1, keepdims=True)

    # ---- L2: fp8 sims + per-shard top-8-per-tile candidates, bank-sharded --
    qT8 = np.ascontiguousarray(q.T).astype(FP8)   # (D, B)
    bankT8 = mb.T.astype(FP8)                     # (D, M), C-contig via astype
    l2 = _get("l2")
    in_maps = [
        {"qT": qT8,
         "bankT": np.ascontiguousarray(bankT8[:, i * ML:(i + 1) * ML])}
        for i in range(NC)
    ]
    r2 = _run(l2, in_maps, "l2")
    vals = np.stack([np.asarray(r2[i]["tv"], np.float32) for i in range(NC)],
                    axis=0)                                    # (NC, B, NCAND)
    lidx = np.stack([r2[i]["ti"] for i in range(NC)], axis=0).astype(np.int64)

    # Each slot's position survived an 8->1 fold, so its row is one of 8
    # (pos + k*512 within its tile; the short last tile has just itself).
    # The fp8 sims only steer which slots enter the pool; a top-32 pool per
    # query is expanded to its <=128 possible rows, rescored exactly in
    # fp32, deduped, re-filtered, and the exact top-16 kept (measured rel
    # err ~2.2e-3 end to end).
    tile_of = np.arange(NCAND, dtype=np.int64) // 8
    tile_c0 = np.array([c for c, _ in L2_EMIT], dtype=np.int64)[tile_of]
    tile_cw = np.array([w for _, w in L2_EMIT], dtype=np.int64)[tile_of]
    core_off = (np.arange(NC, dtype=np.int64) * ML)[:, None, None]

    valid = (vals >= -0.02) & (vals <= 0.9995)
    mvals = np.where(valid, vals, -np.inf)
    flat_v = np.transpose(mvals, (1, 0, 2)).reshape(B, NC * NCAND)
    # per-slot metadata aligned with flat_v columns
    slot_pos = np.transpose(lidx, (1, 0, 2)).reshape(B, NC * NCAND)
    slot_c0 = np.broadcast_to(
        (tile_c0[None, :] + core_off[:, 0]).reshape(1, NC * NCAND),
        (B, NC * NCAND))
    slot_cw = np.broadcast_to(
        np.broadcast_to(tile_cw[None, :], (NC, NCAND)).reshape(1, NC * NCAND),
        (B, NC * NCAND))

    part = np.argpartition(-flat_v, POOL - 1, axis=1)[:, :POOL]
    pool_v = np.take_along_axis(flat_v, part, axis=1)          # (B, POOL)
    pool_pos = np.take_along_axis(slot_pos, part, axis=1)
    pool_c0 = np.take_along_axis(slot_c0, part, axis=1)
    pool_cw = np.take_along_axis(slot_cw, part, axis=1)
    ks = np.arange(8, dtype=np.int64) * 512
    cand = pool_c0[:, :, None] + (
        (pool_pos[:, :, None] + ks[None, None, :]) % pool_cw[:, :, None])
    cand = cand.reshape(B, POOL * 8)                           # (B, 192)
    cand_ok = np.repeat(np.isfinite(pool_v), 8, axis=1)
    cand = np.where(cand_ok, cand, 0)
    cv = np.einsum("bd,bkd->bk", q, mb[cand], optimize=True)   # exact fp32
    cv = np.where(cand_ok & (cv >= THRESH) & (cv <= 0.999), cv, -np.inf)
    # dedup repeated rows (fold ties / short-tile wrap) so the 16 retrieved
    # rows are distinct like the reference's top_k
    srt = np.argsort(cand, axis=1, kind="stable")
    cs = np.take_along_axis(cand, srt, axis=1)
    dup = np.zeros_like(cand_ok)
    dup[:, 1:] = cs[:, 1:] == cs[:, :-1]
    dup_unsrt = np.zeros_like(dup)
    np.put_along_axis(dup_unsrt, srt, dup, axis=1)
    cv = np.where(dup_unsrt, -np.inf, cv)
    sel = np.argsort(-cv, axis=1, kind="stable")[:, :TOPK]
    topv = np.take_along_axis(cv, sel, axis=1)                 # (B, 16) exact
    topi = np.take_along_axis(cand, sel, axis=1)

    if not np.any(topv > -np.inf):
        # nothing retrieved anywhere -> output == hidden_states exactly
        return hs.copy()

    topv_dev = np.where(np.isfinite(topv), topv, NEG).astype(np.float32)
    topi = np.where(np.isfinite(topv), topi, 0)

    # ---- L3: attention + gate + LN + broadcast add, batch-sharded ----
    bankT16 = mb.T.astype(np.float16)             # (D, M)
    wall16 = np.concatenate(
        [Wq.T.astype(np.float16), Wk.T.astype(np.float16),
         Wv.T.astype(np.float16), Wo.T.astype(np.float16)], axis=0)  # (4D, D)
    seriesT16 = series.T.astype(np.float16)       # (D, B)
    wgs, wgm = Wg[0, :D], Wg[0, D:]
    bqk_h = np.concatenate(
        [bq.reshape(4, 128).T, bk.reshape(4, 128).T,
         bv.reshape(4, 128).T], axis=1)                        # (128, 12)
    bqk_h = np.ascontiguousarray(bqk_h, np.float32)
    l3 = _get("l3")
    in_maps = []
    for i in range(NC):
        sl = slice(i * BL, (i + 1) * BL)
        idx_flat = topi[sl].reshape(-1)  # (BL*16,)
        strT = np.concatenate(
            [seriesT16[:, sl], bankT16[:, idx_flat]], axis=1)  # (D, BL+R)
        vecp = np.stack(
            [np.broadcast_to(bo, (BL, D)),
             np.broadcast_to(wgs, (BL, D)), np.broadcast_to(wgm, (BL, D)),
             np.broadcast_to(ln_g, (BL, D)), np.broadcast_to(ln_b, (BL, D))],
            axis=1).astype(np.float16)                         # (BL, 5, D)
        tiny = np.concatenate(
            [topv_dev[sl], np.full((BL, 1), bg[0], np.float32)], axis=1)
        vmask = topv_dev[sl] > -1.0e30                          # (BL, 16)
        rowp = np.concatenate(
            [np.where(vmask, 0.0, NEG).reshape(-1),
             vmask.astype(np.float32).reshape(-1)]).astype(np.float32)
        in_maps.append({
            "hid": hs16[sl],
            "wall": wall16,
            "strT": np.ascontiguousarray(strT),
            "series": np.ascontiguousarray(series[sl], np.float32),
            "vecp": np.ascontiguousarray(vecp),
            "tiny": np.ascontiguousarray(tiny, np.float32),
            "rowp": rowp,
            "bqk": bqk_h,
        })
    r3 = _run(l3, in_maps, "l3")
    out = np.empty((B, S, D), np.float32)
    for i in range(NC):
        out[i * BL:(i + 1) * BL] = r3[i]["out"]
    return out
